# revision 6
# baseline (speedup 1.0000x reference)
"""Trainium2 Bass kernel for nn_Attention_54142357733562 (linear/sparse attention).

Reference math (per batch b, with x flattened to [C, N]):
    Q = wq @ x ; K = wk @ x ; V = wv @ x            (1x1 convs, + zero biases)
    Qn = Q / ||Q||_c ; Kn = K / ||K||_c             (L2 norm over channel dim)
    k_sum = sum_n Kn + EPS                          [Cqk]
    tailor = 1 / (N + Qn^T k_sum)                   [N]
    kv = Kn V^T                                     [Cqk, C]
    out = (value_sum + kv^T Qn) * tailor            [C, N]

Key reformulation: V is NEVER materialized. Since V = wv x,
    kv        = Kn x^T wv^T      -> accumulate kx^T = x^T [Kn | 1]  (per shard)
    value_sum = wv x_sum          (x_sum rides along as kx^T's ones column)
so phase 1 only projects Q,K (64 channels instead of 320), PE-transposes x
tiles ([c,n] -> [n,c]) and accumulates three flipped matmuls per sub-tile
(rhs = [Kn|1], 33 cols each: two x^T c-chunks + a ones row for k_sum) into a
single PSUM bank. The AllReduce payload [128, 99] carries kx^T | x_sum |
k_sum; a once-per-iteration epilogue computes kvp = [kx;x_sum]^T wv^T via two
256-col matmuls. Phase 2 (out = kvp^T Q'' with per-n scale folded) unchanged.

PE cost/sub-tile: 128 (QK) + 256 (x-transpose) + 99 (kx) + 320 (phase 2)
= ~806 cycles vs ~1218 before -> ~86us/iter PE at 2.4 GHz, just under the
~93us DMA floor (in+out share the 360 GB/s DMA bus) -> DMA-bound ridge.

Collective latency hiding: everything downstream of the AllReduce (epilogue
matmuls, ksum) is NOT emitted at the iteration tail -- any engine op waiting
on the collective would block that engine's in-order queue. Instead the
epilogue is deferred and spliced into the NEXT iteration's instruction stream
after `skew` macros (~23us), by which time the AllReduce has landed. Phase 2
of iteration i-1 likewise interleaves into iteration i's phase 1, paced at
1-2 chunks per macro so it finishes in-loop (no serial flush bubble).

Sharding: 8 cores = 4 batches x 2 N-halves; AllReduce over each pair.
"""

import numpy as np
import ml_dtypes
from contextlib import ExitStack

import concourse.bass as bass
import concourse.mybir as mybir
import concourse.tile as tile
from concourse import bacc
from concourse.bass_utils import run_bass_kernel_spmd
from concourse.masks import make_identity

F32 = mybir.dt.float32
F32R = mybir.dt.float32r
BF16 = mybir.dt.bfloat16


def _mdt(mm_dtype):
    return {"f32r": F32R, "f32": F32, "bf16": BF16}[mm_dtype]


def _np_io(mm_dtype):
    return ml_dtypes.bfloat16 if mm_dtype == "bf16" else np.float32


C = 256
CQK = 32
J = 2 * CQK + C  # 320 = stacked [Q|K|V] weight columns (V part used in epilogue)
EPS = 1e-6
P = 128
NT = 1024  # macro-tile width along N
ST = NT // P  # 8 sub-tiles per macro

# stash row layout (W=68): [Q 0:32 | s 32 | ||K|| 33 | K 34:66 | pad]
SW = 68
KXW = 3 * (CQK + 1)  # 99: [kxT c0:128 | kxT c128:256 | ksum row] + x_sum/count


def build_attention_nc(nsh, n_total, num_cores, groups, mm_dtype="bf16",
                       repeat=1, use_collective=True, phases=(1, 2), skew=8):
    nc = bacc.Bacc("TRN2", target_bir_lowering=False, debug=False,
                   num_devices=num_cores)
    MDT = _mdt(mm_dtype)
    ODT = BF16 if mm_dtype == "bf16" else F32
    PSDT = F32
    xs = nc.dram_tensor("xs", [C, nsh], MDT, kind="ExternalInput").ap()
    wt = nc.dram_tensor("wt", [C, J], MDT, kind="ExternalInput").ap()
    out = nc.dram_tensor("out", [C, nsh], ODT, kind="ExternalOutput").ap()

    NM = nsh // NT
    SROW = nsh // P
    HNT = NT // 2
    CS = CQK + 1  # 33

    xs_r = xs.rearrange("(o p) n -> p o n", p=P)  # [128, 2, nsh]
    out_r = out.rearrange("(o p) n -> p o n", p=P)
    wt_r = wt.rearrange("(o p) j -> p o j", p=P)  # [128, 2, 320]

    mult = mybir.AluOpType.mult

    def qk_split(ap_3d, width):
        """[P, rows, SW] slice -> [P, rows, 2, width] view of the Q and K
        column groups (offsets 0 and CQK+2)."""
        return bass.AP(
            tensor=ap_3d.tensor,
            offset=ap_3d.offset,
            ap=[ap_3d.ap[0], ap_3d.ap[1], [CQK + 2, 2], [1, width]],
        )

    with tile.TileContext(nc) as tc, ExitStack() as ctx:
        consts = ctx.enter_context(tc.tile_pool(name="consts", bufs=1))
        stashp = ctx.enter_context(tc.tile_pool(name="stashp", bufs=2))
        xferp = ctx.enter_context(tc.tile_pool(name="xferp", bufs=2))
        dram = ctx.enter_context(tc.tile_pool(name="dram", bufs=2, space="DRAM"))
        xp = ctx.enter_context(tc.tile_pool(name="xp", bufs=8))
        xtsp = ctx.enter_context(tc.tile_pool(name="xtsp", bufs=4))
        kvb = ctx.enter_context(tc.tile_pool(name="kvb", bufs=4))
        scr = ctx.enter_context(tc.tile_pool(name="scr", bufs=2))
        scr2 = ctx.enter_context(tc.tile_pool(name="scr2", bufs=3))
        qscp = ctx.enter_context(tc.tile_pool(name="qscp", bufs=4))
        qtp = ctx.enter_context(tc.tile_pool(name="qtp", bufs=3))
        outp = ctx.enter_context(tc.tile_pool(name="outp", bufs=3))
        # PSUM banks: qk 1 + xq 2 + kx 1 + qt 1 + out 3 = 8
        ps_qk = ctx.enter_context(tc.tile_pool(name="ps_qk", bufs=1, space="PSUM"))
        ps_xq = ctx.enter_context(tc.tile_pool(name="ps_xq", bufs=2, space="PSUM"))
        ps_kx = ctx.enter_context(tc.tile_pool(name="ps_kx", bufs=1, space="PSUM"))
        ps_qt = ctx.enter_context(tc.tile_pool(name="ps_qt", bufs=1, space="PSUM"))
        ps_out = ctx.enter_context(tc.tile_pool(name="ps_out", bufs=3, space="PSUM"))

        wsb = consts.tile([P, 2, J], MDT)
        nc.sync.dma_start(wsb, wt_r)
        ident = consts.tile([P, P], F32)
        make_identity(nc, ident)
        ones_r = consts.tile([P, 1], MDT)
        ones_f = consts.tile([P, 1], F32)
        nc.vector.memset(ones_f, 1.0)
        nc.vector.tensor_copy(ones_r, ones_f)
        ident_r = consts.tile([P, P], MDT)
        nc.vector.tensor_copy(ident_r, ident)

        # ---------------- phase 1 ----------------

        def ph1_macro(m, xt_state):
            stash, prev = xt_state["stash"], xt_state["prev"]
            xt = xp.tile([P, 2, NT], MDT, tag="xt")
            nc.sync.dma_start(xt, xs_r[:, :, m * NT:(m + 1) * NT])

            mst = stash[:, m * ST:(m + 1) * ST, :]  # [128, 8, 68]

            # QK projection: 64 channels, all 8 sub-tiles into one PSUM bank
            qs = ps_qk.tile([P, ST, 2 * CQK], PSDT, tag="qs")
            for s in range(ST):
                for o in range(2):
                    nc.tensor.matmul(
                        qs[:, s, :],
                        xt[:, o, s * P:(s + 1) * P],
                        wsb[:, o, 0:2 * CQK],
                        start=(o == 0),
                        stop=(o == 1),
                    )
            # Q,K -> stash in one strided copy (ACT; DVE is the scarcer engine)
            nc.scalar.copy(
                qk_split(mst, CQK),
                qs.rearrange("p s (g c) -> p s g c", g=2))

            # x^T via PE transposes, drains split DVE/ACT
            xts = xtsp.tile([P, ST, C], MDT, tag="xts")
            for h in range(2):
                xq = ps_xq.tile([P, 4, 2, P], MDT, tag="xq")
                for s2 in range(4):
                    s = 4 * h + s2
                    for o in range(2):
                        nc.tensor.transpose(
                            xq[:, s2, o, :], xt[:, o, s * P:(s + 1) * P],
                            ident_r)
                if h == 0:
                    nc.vector.tensor_copy(
                        xts[:, 0:4, :], xq.rearrange("p s o c -> p s (o c)"))
                else:
                    nc.scalar.copy(
                        xts[:, 4:8, :], xq.rearrange("p s o c -> p s (o c)"))

            # normalization chain (squares/scales on POOL, reduces on DVE)
            sq = scr.tile([P, ST, 2, CQK], MDT, tag="sq")
            nc.gpsimd.tensor_tensor(sq, qk_split(mst, CQK),
                                    qk_split(mst, CQK), mult)
            ssq = scr.tile([P, ST, 2], F32, tag="ssq")
            nc.vector.reduce_sum(ssq, sq, axis=mybir.AxisListType.X)
            nc.scalar.sqrt(mst[:, :, CQK:CQK + 2], ssq)
            rkn = scr.tile([P, ST, 1], F32, tag="rkn")
            nc.vector.reciprocal(rkn, mst[:, :, CQK + 1:CQK + 2])
            kvt = kvb.tile([P, ST, CS], MDT, tag="kvt")
            nc.gpsimd.tensor_tensor(kvt[:, :, 0:CQK],
                                    mst[:, :, CQK + 2:CQK + 2 + CQK],
                                    rkn.to_broadcast((P, ST, CQK)), mult)
            nc.gpsimd.memset(kvt[:, :, CQK:CS], 1.0)

            # kx accumulation runs TWO macros behind (drain/normalize slack)
            if len(prev) == 2:
                kx_mms(*prev.pop(0), xt_state)
            prev.append((m, kvt, xts))

        def kx_mms(mm, kvt_mm, xts_mm, xt_state):
            kx_acc = xt_state["kx_acc"]
            for s in range(ST):
                st0 = (mm == 0 and s == 0)
                sp0 = (mm == NM - 1 and s == ST - 1)
                nc.tensor.matmul(kx_acc[0:P, 0:CS],
                                 xts_mm[:, s, 0:P], kvt_mm[:, s, :],
                                 start=st0, stop=sp0)
                nc.tensor.matmul(kx_acc[0:P, CS:2 * CS],
                                 xts_mm[:, s, P:C], kvt_mm[:, s, :],
                                 start=st0, stop=sp0)
                nc.tensor.matmul(kx_acc[0:1, 2 * CS:KXW],
                                 ones_r[:, 0:1], kvt_mm[:, s, :],
                                 start=st0, stop=sp0)

        def ph1_tail_post(xt_state):
            """Flush kx matmuls, post the AllReduce. No collective-dependent
            engine ops here -- they'd block the engines' in-order queues."""
            for pp in xt_state["prev"]:
                kx_mms(*pp, xt_state)
            xt_state["prev"] = []

            kx_sb = xferp.tile([P, KXW], F32, tag="kx_sb")
            nc.gpsimd.memset(kx_sb, 0.0)
            nc.vector.tensor_copy(kx_sb[:, 0:2 * CS],
                                  xt_state["kx_acc"][0:P, 0:2 * CS])
            nc.vector.tensor_copy(kx_sb[0:1, 2 * CS:KXW],
                                  xt_state["kx_acc"][0:1, 2 * CS:KXW])
            cc_in = dram.tile([P, KXW], F32, tag="cc_in")
            cc_out = dram.tile([P, KXW], F32, tag="cc_out")
            nc.sync.dma_start(cc_in, kx_sb)
            if use_collective:
                nc.gpsimd.collective_compute(
                    "AllReduce",
                    mybir.AluOpType.add,
                    replica_groups=groups,
                    ins=[cc_in.opt()],
                    outs=[cc_out.opt()],
                )
            else:
                nc.sync.dma_start(cc_out, cc_in)
            return cc_out

        def tail_finish(cc_out):
            """Collective-dependent epilogue: kvp = [kx;x_sum]^T wv^T, ksum.
            Emitted `skew` macros into the NEXT iteration."""
            kxb_f32 = xferp.tile([P, 2, CS], F32, tag="kxb_f32")
            nc.sync.dma_start(
                kxb_f32, cc_out[:, 0:2 * CS].rearrange("p (o c) -> p o c", o=2))
            kxb = xferp.tile([P, 2, CS], MDT, tag="kxb")
            nc.vector.tensor_copy(kxb, kxb_f32)
            ksum = xferp.tile([P, CS], F32, tag="ksum")
            nc.sync.dma_start(ksum[:, 0:CQK],
                              cc_out[0:1, 2 * CS:2 * CS + CQK]
                              .partition_broadcast(P))
            nc.vector.tensor_scalar_add(ksum[:, 0:CQK], ksum[:, 0:CQK], EPS)
            nc.vector.memset(ksum[:, CQK:CS], float(n_total))
            kv_ps = ps_out.tile([P, HNT], PSDT, tag="out")
            for o in range(2):
                nc.tensor.matmul(kv_ps[0:CS, 0:C], kxb[:, o, :],
                                 wsb[:, o, 2 * CQK:J],
                                 start=(o == 0), stop=(o == 1))
            kvp = xferp.tile([CS, C], MDT, tag="kvp")
            nc.vector.tensor_copy(kvp, kv_ps[0:CS, 0:C])
            return kvp, ksum

        # ---------------- phase 2 (one macro-chunk per yield) ----------------

        def ph2_gen(stash, kvp, ksum, first):
            def chain(m):
                st_sl = stash[:, m * ST:(m + 1) * ST, 0:CS]
                prod = scr2.tile([P, ST, CS], F32, tag="prod")
                nc.gpsimd.tensor_tensor(
                    prod, st_sl,
                    ksum[:, None, :].to_broadcast((P, ST, CS)), mult)
                den = scr2.tile([P, ST, 1], F32, tag="den")
                nc.vector.reduce_sum(den, prod, axis=mybir.AxisListType.X)
                d = scr2.tile([P, ST, 1], F32, tag="d")
                nc.vector.reciprocal(d, den)
                # rows padded to 64 so transposed PAIRS land on psum
                # partitions 0 and 64. col 33 picks up ||K||*d: finite junk.
                qsc = qscp.tile([P, ST, 2 * CQK], MDT, tag="qsc")
                if first and m < 4:  # first rotation: make pad finite
                    nc.gpsimd.memset(qsc[:, :, CQK + 2:2 * CQK], 0.0)
                nc.gpsimd.tensor_tensor(
                    qsc[:, :, 0:CQK + 2],
                    stash[:, m * ST:(m + 1) * ST, 0:CQK + 2],
                    d.to_broadcast((P, ST, CQK + 2)), mult)
                return qsc

            def mms(m, qsc):
                qt_ps = ps_qt.tile([P, ST // 2, P], MDT, tag="qt_ps")
                for j in range(ST // 2):
                    pair = qsc[:, 2 * j:2 * j + 2, :].rearrange(
                        "p a b -> p (a b)")  # [128, 128]
                    nc.tensor.transpose(qt_ps[:, j, :], pair, ident_r)
                qt_sb = qtp.tile([CS, ST, P], MDT, tag="qt_sb")
                nc.vector.tensor_copy(qt_sb[:, 0::2, :],
                                      qt_ps[0:CS, :, :])
                nc.vector.tensor_copy(qt_sb[:, 1::2, :],
                                      qt_ps[2 * CQK:2 * CQK + CS, :, :])
                # 4 single-bank out matmuls; drains interleaved so the 4th
                # mm only waits on the 1st drain (pool bufs=3)
                ot = outp.tile([P, 2, NT], ODT, tag="ot")
                for i, (nh, blk) in enumerate(
                        ((0, 0), (0, 1), (1, 0), (1, 1))):
                    o_ps = ps_out.tile([P, HNT], PSDT, tag="out")
                    nc.tensor.matmul(
                        o_ps,
                        kvp[:, blk * P:(blk + 1) * P],
                        qt_sb[:, nh * (ST // 2):(nh + 1) * (ST // 2), :],
                        start=True,
                        stop=True,
                    )
                    dst = ot[:, blk, nh * HNT:(nh + 1) * HNT]
                    if i == 0:
                        nc.vector.tensor_copy(dst, o_ps)
                    else:
                        nc.scalar.copy(dst, o_ps)
                return ot

            def drains(m, ot):
                nc.sync.dma_start(out_r[:, :, m * NT:(m + 1) * NT], ot)

            mmed = []
            for m in range(NM):
                qsc = chain(m)
                mmed.append((m, qsc))
                if len(mmed) == 2:
                    mm_m, qsc_mm = mmed.pop(0)
                    drains(mm_m, mms(mm_m, qsc_mm))
                yield
            for mm_m, qsc_mm in mmed:
                drains(mm_m, mms(mm_m, qsc_mm))
            yield

        # ---------------- pipelined repeat loop ----------------
        gen = None
        gen_left = 0
        pend_cc = None
        prev_stash = None
        prev_first = False
        for it in range(repeat):
            stash = stashp.tile([P, SROW, SW], MDT, tag="stash")
            kx_acc = ps_kx.tile([P, 512], F32, tag="kx")
            xt_state = {"stash": stash, "prev": [], "kx_acc": kx_acc}
            for m in range(NM):
                ph1_macro(m, xt_state)
                if pend_cc is not None and m == skew - 1:
                    kvp, ksum = tail_finish(pend_cc)
                    pend_cc = None
                    gen = ph2_gen(prev_stash, kvp, ksum, first=prev_first)
                    gen_left = NM + 1
                    gen_ready_m = m + 2
                if gen is not None and m >= gen_ready_m:
                    slots_left = NM - 1 - m
                    pulls = 1 if gen_left <= slots_left else 2
                    for _ in range(pulls):
                        if next(gen, StopIteration) is StopIteration:
                            gen = None
                            break
                        gen_left -= 1
            cc_out = ph1_tail_post(xt_state)
            if gen is not None:  # shouldn't happen with pacing, but be safe
                for _ in gen:
                    pass
                gen = None
            if 2 in phases:
                pend_cc = cc_out
                prev_stash = stash
                prev_first = (it == 0)
            else:
                src = xs_r[:, :, 0:NT]
                if MDT != ODT:
                    src = src.bitcast(ODT)
                nc.sync.dma_start(out_r[:, :, 0:NT], src)
        if pend_cc is not None:
            kvp, ksum = tail_finish(pend_cc)
            for _ in ph2_gen(prev_stash, kvp, ksum, first=prev_first):
                pass

    nc.compile()
    return nc


_NC_CACHE = {}


def _get_nc(nsh, n_total, num_cores, groups_key, mm_dtype="bf16"):
    key = (nsh, n_total, num_cores, groups_key, mm_dtype)
    if key not in _NC_CACHE:
        groups = [list(g) for g in groups_key]
        _NC_CACHE[key] = build_attention_nc(nsh, n_total, num_cores, groups,
                                            mm_dtype)
    return _NC_CACHE[key]


def _kernel_numpy(x, wq, bq, wk, bk, wv, bv):
    """Plain numpy fallback (used only for nonzero biases / odd shapes)."""
    b, c, h, w = x.shape
    n = h * w
    xf = x.reshape(b, c, n).astype(np.float64)
    Q = np.einsum("oc,bcn->bon", wq.astype(np.float64), xf) + bq.astype(np.float64)[None, :, None]
    K = np.einsum("oc,bcn->bon", wk.astype(np.float64), xf) + bk.astype(np.float64)[None, :, None]
    V = np.einsum("oc,bcn->bon", wv.astype(np.float64), xf) + bv.astype(np.float64)[None, :, None]
    Qn = Q / np.linalg.norm(Q, axis=1, keepdims=True)
    Kn = K / np.linalg.norm(K, axis=1, keepdims=True)
    k_sum = Kn.sum(-1) + EPS
    tailor = 1.0 / (n + np.einsum("bmn,bm->bn", Qn, k_sum))
    value_sum = V.sum(-1)
    kv = np.einsum("bmn,bcn->bmc", Kn, V)
    ms = value_sum[:, :, None] + np.einsum("bmn,bmc->bcn", Qn, kv)
    return (ms * tailor[:, None, :]).reshape(b, c, h, w).astype(np.float32)


def kernel(x, wq, bq, wk, bk, wv, bv):
    x = np.asarray(x, dtype=np.float32)
    B, Cc, H, W = x.shape
    if (any(np.any(np.asarray(b_) != 0) for b_ in (bq, bk, bv))
            or Cc != C or wq.shape != (CQK, C) or wv.shape != (C, C)
            or (H * W) % (2 * NT) != 0 or B != 4):
        return _kernel_numpy(x, wq, bq, wk, bk, wv, bv)
    N = H * W
    ncores = 8
    shards_per_batch = ncores // B  # 2
    nsh = N // shards_per_batch  # 32768
    groups_key = tuple(
        tuple(range(b * shards_per_batch, (b + 1) * shards_per_batch))
        for b in range(B)
    )

    mm_dtype = "bf16"
    io_t = _np_io(mm_dtype)
    wt = np.ascontiguousarray(
        np.concatenate([np.asarray(wq).T, np.asarray(wk).T, np.asarray(wv).T],
                       axis=1).astype(io_t))

    nc = _get_nc(nsh, N, ncores, groups_key, mm_dtype)

    xr = x.reshape(B, Cc, N).astype(io_t)
    in_maps = []
    for core in range(ncores):
        b, hh = core // shards_per_batch, core % shards_per_batch
        in_maps.append({
            "xs": np.ascontiguousarray(xr[b, :, hh * nsh:(hh + 1) * nsh]),
            "wt": wt,
        })

    res = run_bass_kernel_spmd(nc, in_maps, list(range(ncores)))

    out = np.empty((B, Cc, N), np.float32)
    for core in range(ncores):
        b, hh = core // shards_per_batch, core % shards_per_batch
        out[b, :, hh * nsh:(hh + 1) * nsh] = np.asarray(
            res.results[core]["out"]).astype(np.float32)
    return out.reshape(B, Cc, H, W)


# revision 21
# speedup vs baseline: 1.1122x; 1.1122x over previous
"""Trainium2 Bass kernel for nn_Attention_54142357733562 (linear/sparse attention).

Reference math (per batch b, with x flattened to [C, N]):
    Q = wq @ x ; K = wk @ x ; V = wv @ x            (1x1 convs, + zero biases)
    Qn = Q / ||Q||_c ; Kn = K / ||K||_c             (L2 norm over channel dim)
    k_sum = sum_n Kn + EPS                          [Cqk]
    tailor = 1 / (N + Qn^T k_sum)                   [N]
    kv = Kn V^T                                     [Cqk, C]
    out = (value_sum + kv^T Qn) * tailor            [C, N]

Key reformulation: V is NEVER materialized. Since V = wv x,
    kv        = Kn x^T wv^T      -> accumulate kx^T = x^T [Kn | 1]  (per shard)
    value_sum = wv x_sum          (x_sum rides along as kx^T's ones column)
so phase 1 only projects Q,K (64 channels instead of 320), PE-transposes x
tiles ([c,n] -> [n,c]) and accumulates three flipped matmuls per sub-tile
(rhs = [Kn|1], 33 cols each: two x^T c-chunks + a ones row for k_sum) into a
single PSUM bank. The AllReduce payload [128, 99] carries kx^T | x_sum |
k_sum; a once-per-iteration epilogue computes kvp = [kx;x_sum]^T wv^T via two
256-col matmuls. Phase 2 (out = kvp^T Q'' with per-n scale folded) unchanged.

PE cost/sub-tile: 128 (QK) + 256 (x-transpose) + 99 (kx) + 320 (phase 2)
= ~806 cycles vs ~1218 before -> ~86us/iter PE at 2.4 GHz, just under the
~93us DMA floor (in+out share the 360 GB/s DMA bus) -> DMA-bound ridge.

Collective latency hiding: everything downstream of the AllReduce (epilogue
matmuls, ksum) is NOT emitted at the iteration tail -- any engine op waiting
on the collective would block that engine's in-order queue. Instead the
epilogue is deferred and spliced into the NEXT iteration's instruction stream
after `skew` macros (~23us), by which time the AllReduce has landed. Phase 2
of iteration i-1 likewise interleaves into iteration i's phase 1, paced at
1-2 chunks per macro so it finishes in-loop (no serial flush bubble).

Sharding: 8 cores = 4 batches x 2 N-halves; AllReduce over each pair.
"""

import numpy as np
import ml_dtypes
from contextlib import ExitStack

import concourse.bass as bass
import concourse.mybir as mybir
import concourse.tile as tile
from concourse import bacc
from concourse.bass_utils import run_bass_kernel_spmd
from concourse.masks import make_identity

F32 = mybir.dt.float32
F32R = mybir.dt.float32r
BF16 = mybir.dt.bfloat16


def _mdt(mm_dtype):
    return {"f32r": F32R, "f32": F32, "bf16": BF16}[mm_dtype]


def _np_io(mm_dtype):
    return ml_dtypes.bfloat16 if mm_dtype == "bf16" else np.float32


C = 256
CQK = 32
J = 2 * CQK + C  # 320 = stacked [Q|K|V] weight columns (V part used in epilogue)
EPS = 1e-6
P = 128
NT = 1024  # macro-tile width along N
ST = NT // P  # 8 sub-tiles per macro

# stash row layout (W=68): [Q 0:32 | s 32 | ||K|| 33 | K 34:66 | pad]
SW = 68
KXW = 3 * (CQK + 1)  # 99: [kxT c0:128 | kxT c128:256 | ksum row] + x_sum/count


def build_attention_nc(nsh, n_total, num_cores, groups, mm_dtype="bf16",
                       repeat=1, use_collective=True, phases=(1, 2), skew=8,
                       dbg=False):
    nc = bacc.Bacc("TRN2", target_bir_lowering=False, debug=False,
                   num_devices=num_cores)
    MDT = _mdt(mm_dtype)
    ODT = BF16 if mm_dtype == "bf16" else F32
    PSDT = F32
    xs = nc.dram_tensor("xs", [C, nsh], MDT, kind="ExternalInput").ap()
    wt = nc.dram_tensor("wt", [C, J], MDT, kind="ExternalInput").ap()
    out = nc.dram_tensor("out", [C, nsh], ODT, kind="ExternalOutput").ap()
    if dbg:
        dbg_cc = nc.dram_tensor("dbg_cc", [P, 512], F32,
                                kind="ExternalOutput").ap()
        dbg_kvp = nc.dram_tensor("dbg_kvp", [CQK + 1, C], MDT,
                                 kind="ExternalOutput").ap()
        dbg_ksum = nc.dram_tensor("dbg_ksum", [P, CQK + 1], F32,
                                  kind="ExternalOutput").ap()
        dbg_xts = nc.dram_tensor("dbg_xts", [2, P, ST, C], BF16,
                                 kind="ExternalOutput").ap()
        dbg_kxsb = nc.dram_tensor("dbg_kxsb", [P, 512], F32,
                                  kind="ExternalOutput").ap()

    NM = nsh // NT
    SROW = nsh // P
    HNT = NT // 2
    CS = CQK + 1  # 33

    xs_r = xs.rearrange("(o p) n -> p o n", p=P)  # [128, 2, nsh]
    out_r = out.rearrange("(o p) n -> p o n", p=P)
    wt_r = wt.rearrange("(o p) j -> p o j", p=P)  # [128, 2, 320]

    mult = mybir.AluOpType.mult

    def qk_split(ap_3d, width):
        """[P, rows, SW] slice -> [P, rows, 2, width] view of the Q and K
        column groups (offsets 0 and CQK+2)."""
        return bass.AP(
            tensor=ap_3d.tensor,
            offset=ap_3d.offset,
            ap=[ap_3d.ap[0], ap_3d.ap[1], [CQK + 2, 2], [1, width]],
        )

    with tile.TileContext(nc) as tc, ExitStack() as ctx:
        consts = ctx.enter_context(tc.tile_pool(name="consts", bufs=1))
        stashp = ctx.enter_context(tc.tile_pool(name="stashp", bufs=2))
        xferp = ctx.enter_context(tc.tile_pool(name="xferp", bufs=2))
        dram = ctx.enter_context(tc.tile_pool(name="dram", bufs=2, space="DRAM"))
        xp = ctx.enter_context(tc.tile_pool(name="xp", bufs=8))
        xtsp = ctx.enter_context(tc.tile_pool(name="xtsp", bufs=4))
        kvb = ctx.enter_context(tc.tile_pool(name="kvb", bufs=4))
        scr = ctx.enter_context(tc.tile_pool(name="scr", bufs=2))
        scr2 = ctx.enter_context(tc.tile_pool(name="scr2", bufs=3))
        qscp = ctx.enter_context(tc.tile_pool(name="qscp", bufs=4))
        qtp = ctx.enter_context(tc.tile_pool(name="qtp", bufs=3))
        outp = ctx.enter_context(tc.tile_pool(name="outp", bufs=3))
        # PSUM banks: qk 1 + xq 2 + kx 1 + qt 1 + out 3 = 8
        ps_qk = ctx.enter_context(tc.tile_pool(name="ps_qk", bufs=1, space="PSUM"))
        ps_xq = ctx.enter_context(tc.tile_pool(name="ps_xq", bufs=2, space="PSUM"))
        ps_kx = ctx.enter_context(tc.tile_pool(name="ps_kx", bufs=1, space="PSUM"))
        ps_qt = ctx.enter_context(tc.tile_pool(name="ps_qt", bufs=1, space="PSUM"))
        ps_out = ctx.enter_context(tc.tile_pool(name="ps_out", bufs=3, space="PSUM"))

        wsb = consts.tile([P, 2, J], MDT)
        nc.sync.dma_start(wsb, wt_r)
        ident = consts.tile([P, P], F32)
        make_identity(nc, ident)
        ones_r = consts.tile([P, 1], MDT)
        ones_f = consts.tile([P, 1], F32)
        nc.vector.memset(ones_f, 1.0)
        nc.vector.tensor_copy(ones_r, ones_f)
        ident_r = consts.tile([P, P], MDT)
        nc.vector.tensor_copy(ident_r, ident)

        # ---------------- phase 1 ----------------

        def ph1_macro(m, xt_state):
            stash, prev = xt_state["stash"], xt_state["prev"]
            xt = xp.tile([P, 2, NT], MDT, tag="xt")
            nc.sync.dma_start(xt, xs_r[:, :, m * NT:(m + 1) * NT])

            mst = stash[:, m * ST:(m + 1) * ST, :]  # [128, 8, 68]

            # QK projection: 64 channels, all 8 sub-tiles into one PSUM bank
            qs = ps_qk.tile([P, ST, 2 * CQK], PSDT, tag="qs")
            for s in range(ST):
                for o in range(2):
                    nc.tensor.matmul(
                        qs[:, s, :],
                        xt[:, o, s * P:(s + 1) * P],
                        wsb[:, o, 0:2 * CQK],
                        start=(o == 0),
                        stop=(o == 1),
                    )
            # Q,K -> stash in one strided copy (ACT; DVE is the scarcer engine)
            nc.scalar.copy(
                qk_split(mst, CQK),
                qs.rearrange("p s (g c) -> p s g c", g=2))

            # x^T via PE transposes, drains split DVE/ACT; col 256 = ones
            xts = xtsp.tile([P, ST, C + 1], MDT, tag="xts")
            nc.gpsimd.memset(xts[:, :, C:C + 1], 1.0)
            for h in range(2):
                xq = ps_xq.tile([P, 4, 2, P], MDT, tag="xq")
                for s2 in range(4):
                    s = 4 * h + s2
                    for o in range(2):
                        nc.tensor.transpose(
                            xq[:, s2, o, :], xt[:, o, s * P:(s + 1) * P],
                            ident_r)
                if h == 0:
                    nc.vector.tensor_copy(
                        xts[:, 0:4, 0:C], xq.rearrange("p s o c -> p s (o c)"))
                else:
                    nc.scalar.copy(
                        xts[:, 4:8, 0:C], xq.rearrange("p s o c -> p s (o c)"))

            # normalization chain (squares/scales on POOL, reduces on DVE)
            sq = scr.tile([P, ST, 2, CQK], MDT, tag="sq")
            nc.gpsimd.tensor_tensor(sq, qk_split(mst, CQK),
                                    qk_split(mst, CQK), mult)
            ssq = scr.tile([P, ST, 2], F32, tag="ssq")
            nc.vector.reduce_sum(ssq, sq, axis=mybir.AxisListType.X)
            nc.scalar.sqrt(mst[:, :, CQK:CQK + 2], ssq)
            rkn = scr.tile([P, ST, 1], F32, tag="rkn")
            nc.vector.reciprocal(rkn, mst[:, :, CQK + 1:CQK + 2])
            kvt = kvb.tile([P, ST, CS], MDT, tag="kvt")
            nc.gpsimd.tensor_tensor(kvt[:, :, 0:CQK],
                                    mst[:, :, CQK + 2:CQK + 2 + CQK],
                                    rkn.to_broadcast((P, ST, CQK)), mult)
            nc.gpsimd.memset(kvt[:, :, CQK:CS], 1.0)

            if dbg and m in (0, 5):
                nc.sync.dma_start(dbg_xts[0 if m == 0 else 1], xts[:, :, 0:C])

            # kx accumulation runs TWO macros behind (drain/normalize slack)
            if len(prev) == 2:
                kx_mms(*prev.pop(0), xt_state)
            prev.append((m, kvt, xts))

        def kx_mms(mm, kvt_mm, xts_mm, xt_state):
            # kx_acc [33, 257] = [Kn|1]^T [xT|1]: rows 0:32 kx (row 32 x_sum),
            # col 256 k_sum (corner = count). Baseline-shaped accumulation
            # (33-col stationary, 257-col moving): small-moving-dim flipped
            # variants showed ~bf16-level per-partial noise on HW.
            kx_acc = xt_state["kx_acc"]
            for s in range(ST):
                nc.tensor.matmul(kx_acc[0:CS, 0:C + 1],
                                 kvt_mm[:, s, :], xts_mm[:, s, :],
                                 start=(mm == 0 and s == 0),
                                 stop=(mm == NM - 1 and s == ST - 1))

        def ph1_tail_post(xt_state):
            """Flush kx matmuls, post the AllReduce. No collective-dependent
            engine ops here -- they'd block the engines' in-order queues."""
            for pp in xt_state["prev"]:
                kx_mms(*pp, xt_state)
            xt_state["prev"] = []

            kx_sb = xferp.tile([CS, C + 1], F32, tag="kx_sb")
            nc.vector.tensor_copy(kx_sb, xt_state["kx_acc"][0:CS, 0:C + 1])
            cc_in = dram.tile([CS, C + 1], F32, tag="cc_in")
            cc_out = dram.tile([CS, C + 1], F32, tag="cc_out")
            nc.sync.dma_start(cc_in, kx_sb)
            if dbg:
                nc.sync.dma_start(dbg_kxsb[0:CS, 0:C + 1], kx_sb)
            if use_collective:
                nc.gpsimd.collective_compute(
                    "AllReduce",
                    mybir.AluOpType.add,
                    replica_groups=groups,
                    ins=[cc_in.opt()],
                    outs=[cc_out.opt()],
                )
            else:
                nc.sync.dma_start(cc_out, cc_in)
            if dbg:
                nc.sync.dma_start(dbg_cc[0:CS, 0:C + 1], cc_out)
            return cc_out

        def tail_finish(cc_out):
            """Collective-dependent epilogue: kvp = [kx;x_sum]^T wv^T, ksum.
            Emitted `skew` macros into the NEXT iteration."""
            kxb_f32 = xferp.tile([CS, C], F32, tag="kxb_f32")
            nc.sync.dma_start(kxb_f32, cc_out[:, 0:C])
            kxb = xferp.tile([CS, C], MDT, tag="kxb")
            nc.vector.tensor_copy(kxb, kxb_f32)
            # transpose [kx; x_sum] -> lhsT layout [c', 33] for the kv matmul
            qt2 = ps_qt.tile([P, ST // 2, P], MDT, tag="qt_ps")
            for o in range(2):
                nc.tensor.transpose(qt2[:, o, 0:CS],
                                    kxb[:, o * P:(o + 1) * P],
                                    ident_r[0:CS, 0:CS])
            kxt = xferp.tile([P, 2, CS], MDT, tag="kxt")
            nc.vector.tensor_copy(kxt, qt2[:, 0:2, 0:CS])
            ksum = xferp.tile([P, CS], F32, tag="ksum")
            nc.sync.dma_start(ksum[:, 0:CQK],
                              cc_out[0:CQK, C:C + 1].partition_broadcast(P))
            nc.vector.tensor_scalar_add(ksum[:, 0:CQK], ksum[:, 0:CQK], EPS)
            nc.vector.memset(ksum[:, CQK:CS], float(n_total))
            kv_ps = ps_out.tile([P, HNT], PSDT, tag="out")
            for o in range(2):
                nc.tensor.matmul(kv_ps[0:CS, 0:C], kxt[:, o, :],
                                 wsb[:, o, 2 * CQK:J],
                                 start=(o == 0), stop=(o == 1))
            kvp = xferp.tile([CS, C], MDT, tag="kvp")
            nc.vector.tensor_copy(kvp, kv_ps[0:CS, 0:C])
            if dbg:
                nc.sync.dma_start(dbg_kvp, kvp)
                nc.sync.dma_start(dbg_ksum, ksum)
            return kvp, ksum

        # ---------------- phase 2 (one macro-chunk per yield) ----------------

        def ph2_gen(stash, kvp, ksum, first):
            def chain(m):
                st_sl = stash[:, m * ST:(m + 1) * ST, 0:CS]
                prod = scr2.tile([P, ST, CS], F32, tag="prod")
                nc.gpsimd.tensor_tensor(
                    prod, st_sl,
                    ksum[:, None, :].to_broadcast((P, ST, CS)), mult)
                den = scr2.tile([P, ST, 1], F32, tag="den")
                nc.vector.reduce_sum(den, prod, axis=mybir.AxisListType.X)
                d = scr2.tile([P, ST, 1], F32, tag="d")
                nc.vector.reciprocal(d, den)
                # rows padded to 64 so transposed PAIRS land on psum
                # partitions 0 and 64. col 33 picks up ||K||*d: finite junk.
                qsc = qscp.tile([P, ST, 2 * CQK], MDT, tag="qsc")
                if first and m < 4:  # first rotation: make pad finite
                    nc.gpsimd.memset(qsc[:, :, CQK + 2:2 * CQK], 0.0)
                nc.gpsimd.tensor_tensor(
                    qsc[:, :, 0:CQK + 2],
                    stash[:, m * ST:(m + 1) * ST, 0:CQK + 2],
                    d.to_broadcast((P, ST, CQK + 2)), mult)
                return qsc

            def mms(m, qsc):
                qt_ps = ps_qt.tile([P, ST // 2, P], MDT, tag="qt_ps")
                for j in range(ST // 2):
                    pair = qsc[:, 2 * j:2 * j + 2, :].rearrange(
                        "p a b -> p (a b)")  # [128, 128]
                    nc.tensor.transpose(qt_ps[:, j, :], pair, ident_r)
                qt_sb = qtp.tile([CS, ST, P], MDT, tag="qt_sb")
                nc.vector.tensor_copy(qt_sb[:, 0::2, :],
                                      qt_ps[0:CS, :, :])
                nc.vector.tensor_copy(qt_sb[:, 1::2, :],
                                      qt_ps[2 * CQK:2 * CQK + CS, :, :])
                # 4 single-bank out matmuls; drains interleaved so the 4th
                # mm only waits on the 1st drain (pool bufs=3)
                ot = outp.tile([P, 2, NT], ODT, tag="ot")
                for i, (nh, blk) in enumerate(
                        ((0, 0), (0, 1), (1, 0), (1, 1))):
                    o_ps = ps_out.tile([P, HNT], PSDT, tag="out")
                    nc.tensor.matmul(
                        o_ps,
                        kvp[:, blk * P:(blk + 1) * P],
                        qt_sb[:, nh * (ST // 2):(nh + 1) * (ST // 2), :],
                        start=True,
                        stop=True,
                    )
                    dst = ot[:, blk, nh * HNT:(nh + 1) * HNT]
                    if i == 0:
                        nc.vector.tensor_copy(dst, o_ps)
                    else:
                        nc.scalar.copy(dst, o_ps)
                return ot

            def drains(m, ot):
                nc.sync.dma_start(out_r[:, :, m * NT:(m + 1) * NT], ot)

            mmed = []
            for m in range(NM):
                qsc = chain(m)
                mmed.append((m, qsc))
                if len(mmed) == 2:
                    mm_m, qsc_mm = mmed.pop(0)
                    drains(mm_m, mms(mm_m, qsc_mm))
                yield
            for mm_m, qsc_mm in mmed:
                drains(mm_m, mms(mm_m, qsc_mm))
            yield

        # ---------------- pipelined repeat loop ----------------
        gen = None
        gen_left = 0
        pend_cc = None
        prev_stash = None
        prev_first = False
        for it in range(repeat):
            stash = stashp.tile([P, SROW, SW], MDT, tag="stash")
            kx_acc = ps_kx.tile([P, 512], F32, tag="kx")
            xt_state = {"stash": stash, "prev": [], "kx_acc": kx_acc}
            for m in range(NM):
                ph1_macro(m, xt_state)
                if pend_cc is not None and m == skew - 1:
                    kvp, ksum = tail_finish(pend_cc)
                    pend_cc = None
                    gen = ph2_gen(prev_stash, kvp, ksum, first=prev_first)
                    gen_left = NM + 1
                    gen_ready_m = m + 2
                if gen is not None and m >= gen_ready_m:
                    slots_left = NM - 1 - m
                    pulls = 1 if gen_left <= slots_left else 2
                    for _ in range(pulls):
                        if next(gen, StopIteration) is StopIteration:
                            gen = None
                            break
                        gen_left -= 1
            cc_out = ph1_tail_post(xt_state)
            if gen is not None:  # shouldn't happen with pacing, but be safe
                for _ in gen:
                    pass
                gen = None
            if 2 in phases:
                pend_cc = cc_out
                prev_stash = stash
                prev_first = (it == 0)
            else:
                src = xs_r[:, :, 0:NT]
                if MDT != ODT:
                    src = src.bitcast(ODT)
                nc.sync.dma_start(out_r[:, :, 0:NT], src)
        if pend_cc is not None:
            kvp, ksum = tail_finish(pend_cc)
            for _ in ph2_gen(prev_stash, kvp, ksum, first=prev_first):
                pass

    nc.compile()
    return nc


_NC_CACHE = {}


def _get_nc(nsh, n_total, num_cores, groups_key, mm_dtype="bf16"):
    key = (nsh, n_total, num_cores, groups_key, mm_dtype)
    if key not in _NC_CACHE:
        groups = [list(g) for g in groups_key]
        _NC_CACHE[key] = build_attention_nc(nsh, n_total, num_cores, groups,
                                            mm_dtype)
    return _NC_CACHE[key]


def _kernel_numpy(x, wq, bq, wk, bk, wv, bv):
    """Plain numpy fallback (used only for nonzero biases / odd shapes)."""
    b, c, h, w = x.shape
    n = h * w
    xf = x.reshape(b, c, n).astype(np.float64)
    Q = np.einsum("oc,bcn->bon", wq.astype(np.float64), xf) + bq.astype(np.float64)[None, :, None]
    K = np.einsum("oc,bcn->bon", wk.astype(np.float64), xf) + bk.astype(np.float64)[None, :, None]
    V = np.einsum("oc,bcn->bon", wv.astype(np.float64), xf) + bv.astype(np.float64)[None, :, None]
    Qn = Q / np.linalg.norm(Q, axis=1, keepdims=True)
    Kn = K / np.linalg.norm(K, axis=1, keepdims=True)
    k_sum = Kn.sum(-1) + EPS
    tailor = 1.0 / (n + np.einsum("bmn,bm->bn", Qn, k_sum))
    value_sum = V.sum(-1)
    kv = np.einsum("bmn,bcn->bmc", Kn, V)
    ms = value_sum[:, :, None] + np.einsum("bmn,bmc->bcn", Qn, kv)
    return (ms * tailor[:, None, :]).reshape(b, c, h, w).astype(np.float32)


def kernel(x, wq, bq, wk, bk, wv, bv):
    x = np.asarray(x, dtype=np.float32)
    B, Cc, H, W = x.shape
    if (any(np.any(np.asarray(b_) != 0) for b_ in (bq, bk, bv))
            or Cc != C or wq.shape != (CQK, C) or wv.shape != (C, C)
            or (H * W) % (2 * NT) != 0 or B != 4):
        return _kernel_numpy(x, wq, bq, wk, bk, wv, bv)
    N = H * W
    ncores = 8
    shards_per_batch = ncores // B  # 2
    nsh = N // shards_per_batch  # 32768
    groups_key = tuple(
        tuple(range(b * shards_per_batch, (b + 1) * shards_per_batch))
        for b in range(B)
    )

    mm_dtype = "bf16"
    io_t = _np_io(mm_dtype)
    wt = np.ascontiguousarray(
        np.concatenate([np.asarray(wq).T, np.asarray(wk).T, np.asarray(wv).T],
                       axis=1).astype(io_t))

    nc = _get_nc(nsh, N, ncores, groups_key, mm_dtype)

    xr = x.reshape(B, Cc, N).astype(io_t)
    in_maps = []
    for core in range(ncores):
        b, hh = core // shards_per_batch, core % shards_per_batch
        in_maps.append({
            "xs": np.ascontiguousarray(xr[b, :, hh * nsh:(hh + 1) * nsh]),
            "wt": wt,
        })

    res = run_bass_kernel_spmd(nc, in_maps, list(range(ncores)))

    out = np.empty((B, Cc, N), np.float32)
    for core in range(ncores):
        b, hh = core // shards_per_batch, core % shards_per_batch
        out[b, :, hh * nsh:(hh + 1) * nsh] = np.asarray(
            res.results[core]["out"]).astype(np.float32)
    return out.reshape(B, Cc, H, W)


# revision 28
# speedup vs baseline: 1.2023x; 1.0810x over previous
"""Trainium2 Bass kernel for nn_Attention_54142357733562 (linear/sparse attention).

Reference math (per batch b, with x flattened to [C, N]):
    Q = wq @ x ; K = wk @ x ; V = wv @ x            (1x1 convs, + zero biases)
    Qn = Q / ||Q||_c ; Kn = K / ||K||_c             (L2 norm over channel dim)
    k_sum = sum_n Kn + EPS                          [Cqk]
    tailor = 1 / (N + Qn^T k_sum)                   [N]
    kv = Kn V^T                                     [Cqk, C]
    out = (value_sum + kv^T Qn) * tailor            [C, N]

Key reformulation: V is NEVER materialized. Since V = wv x,
    kv        = Kn x^T wv^T      -> accumulate kx^T = x^T [Kn | 1]  (per shard)
    value_sum = wv x_sum          (x_sum rides along as kx^T's ones column)
so phase 1 only projects Q,K (64 channels instead of 320), PE-transposes x
tiles ([c,n] -> [n,c]) and accumulates three flipped matmuls per sub-tile
(rhs = [Kn|1], 33 cols each: two x^T c-chunks + a ones row for k_sum) into a
single PSUM bank. The AllReduce payload [128, 99] carries kx^T | x_sum |
k_sum; a once-per-iteration epilogue computes kvp = [kx;x_sum]^T wv^T via two
256-col matmuls. Phase 2 (out = kvp^T Q'' with per-n scale folded) unchanged.

PE cost/sub-tile: 128 (QK) + 256 (x-transpose) + 99 (kx) + 320 (phase 2)
= ~806 cycles vs ~1218 before -> ~86us/iter PE at 2.4 GHz, just under the
~93us DMA floor (in+out share the 360 GB/s DMA bus) -> DMA-bound ridge.

Collective latency hiding: everything downstream of the AllReduce (epilogue
matmuls, ksum) is NOT emitted at the iteration tail -- any engine op waiting
on the collective would block that engine's in-order queue. Instead the
epilogue is deferred and spliced into the NEXT iteration's instruction stream
after `skew` macros (~23us), by which time the AllReduce has landed. Phase 2
of iteration i-1 likewise interleaves into iteration i's phase 1, paced at
1-2 chunks per macro so it finishes in-loop (no serial flush bubble).

Sharding: 8 cores = 4 batches x 2 N-halves; AllReduce over each pair.
"""

import numpy as np
import ml_dtypes
from contextlib import ExitStack

import concourse.bass as bass
import concourse.mybir as mybir
import concourse.tile as tile
from concourse import bacc
from concourse.bass_utils import run_bass_kernel_spmd
from concourse.masks import make_identity

F32 = mybir.dt.float32
F32R = mybir.dt.float32r
BF16 = mybir.dt.bfloat16


def _mdt(mm_dtype):
    return {"f32r": F32R, "f32": F32, "bf16": BF16}[mm_dtype]


def _np_io(mm_dtype):
    return ml_dtypes.bfloat16 if mm_dtype == "bf16" else np.float32


C = 256
CQK = 32
J = 2 * CQK + C  # 320 = stacked [Q|K|V] weight columns (V part used in epilogue)
EPS = 1e-6
P = 128
NT = 1024  # macro-tile width along N
ST = NT // P  # 8 sub-tiles per macro

# stash row layout (W=68): [Q 0:32 | s 32 | ||K|| 33 | K 34:66 | pad]
SW = 68
KXW = 3 * (CQK + 1)  # 99: [kxT c0:128 | kxT c128:256 | ksum row] + x_sum/count


def build_attention_nc(nsh, n_total, num_cores, groups, mm_dtype="bf16",
                       repeat=1, use_collective=True, phases=(1, 2), skew=8,
                       dbg=False):
    nc = bacc.Bacc("TRN2", target_bir_lowering=False, debug=False,
                   num_devices=num_cores)
    MDT = _mdt(mm_dtype)
    ODT = BF16 if mm_dtype == "bf16" else F32
    PSDT = F32
    xs = nc.dram_tensor("xs", [C, nsh], MDT, kind="ExternalInput").ap()
    wt = nc.dram_tensor("wt", [C, J], MDT, kind="ExternalInput").ap()
    out = nc.dram_tensor("out", [C, nsh], ODT, kind="ExternalOutput").ap()
    if dbg:
        dbg_cc = nc.dram_tensor("dbg_cc", [P, 512], F32,
                                kind="ExternalOutput").ap()
        dbg_kvp = nc.dram_tensor("dbg_kvp", [CQK + 1, C], MDT,
                                 kind="ExternalOutput").ap()
        dbg_ksum = nc.dram_tensor("dbg_ksum", [P, CQK + 1], F32,
                                  kind="ExternalOutput").ap()
        dbg_xts = nc.dram_tensor("dbg_xts", [2, P, ST, C], BF16,
                                 kind="ExternalOutput").ap()
        dbg_kxsb = nc.dram_tensor("dbg_kxsb", [P, 512], F32,
                                  kind="ExternalOutput").ap()

    NM = nsh // NT
    SROW = nsh // P
    HNT = NT // 2
    CS = CQK + 1  # 33

    xs_r = xs.rearrange("(o p) n -> p o n", p=P)  # [128, 2, nsh]
    out_r = out.rearrange("(o p) n -> p o n", p=P)
    wt_r = wt.rearrange("(o p) j -> p o j", p=P)  # [128, 2, 320]

    mult = mybir.AluOpType.mult

    def qk_split(ap_3d, width):
        """[P, rows, SW] slice -> [P, rows, 2, width] view of the Q and K
        column groups (offsets 0 and CQK+2)."""
        return bass.AP(
            tensor=ap_3d.tensor,
            offset=ap_3d.offset,
            ap=[ap_3d.ap[0], ap_3d.ap[1], [CQK + 2, 2], [1, width]],
        )

    with tile.TileContext(nc) as tc, ExitStack() as ctx:
        consts = ctx.enter_context(tc.tile_pool(name="consts", bufs=1))
        stashp = ctx.enter_context(tc.tile_pool(name="stashp", bufs=2))
        xferp = ctx.enter_context(tc.tile_pool(name="xferp", bufs=2))
        dram = ctx.enter_context(tc.tile_pool(name="dram", bufs=2, space="DRAM"))
        xp = ctx.enter_context(tc.tile_pool(name="xp", bufs=8))
        xtsp = ctx.enter_context(tc.tile_pool(name="xtsp", bufs=4))
        kvb = ctx.enter_context(tc.tile_pool(name="kvb", bufs=4))
        scr = ctx.enter_context(tc.tile_pool(name="scr", bufs=2))
        scr2 = ctx.enter_context(tc.tile_pool(name="scr2", bufs=3))
        qscp = ctx.enter_context(tc.tile_pool(name="qscp", bufs=4))
        qtp = ctx.enter_context(tc.tile_pool(name="qtp", bufs=3))
        outp = ctx.enter_context(tc.tile_pool(name="outp", bufs=3))
        # PSUM banks: qk/qt shared 2 + xq 2 + kx 1 + out 3 = 8.
        # ps_qk slots serve BOTH ph1's QK psum and ph2's Q''-transpose psum
        # (same tag, same 2KB slot) so each gets double-buffering.
        ps_qk = ctx.enter_context(tc.tile_pool(name="ps_qk", bufs=2, space="PSUM"))
        ps_xq = ctx.enter_context(tc.tile_pool(name="ps_xq", bufs=2, space="PSUM"))
        ps_kx = ctx.enter_context(tc.tile_pool(name="ps_kx", bufs=1, space="PSUM"))
        ps_out = ctx.enter_context(tc.tile_pool(name="ps_out", bufs=3, space="PSUM"))

        wsb = consts.tile([P, 2, J], MDT)
        nc.sync.dma_start(wsb, wt_r)
        ident = consts.tile([P, P], F32)
        make_identity(nc, ident)
        ones_r = consts.tile([P, 1], MDT)
        ones_f = consts.tile([P, 1], F32)
        nc.vector.memset(ones_f, 1.0)
        nc.vector.tensor_copy(ones_r, ones_f)
        ident_r = consts.tile([P, P], MDT)
        nc.vector.tensor_copy(ident_r, ident)

        # ---------------- phase 1 ----------------

        def ph1_macro(m, xt_state):
            stash, prev = xt_state["stash"], xt_state["prev"]
            xt = xp.tile([P, 2, NT], MDT, tag="xt")
            nc.sync.dma_start(xt, xs_r[:, :, m * NT:(m + 1) * NT])

            mst = stash[:, m * ST:(m + 1) * ST, :]  # [128, 8, 68]

            # QK projection: 64 channels, all 8 sub-tiles into one PSUM bank
            qs = ps_qk.tile([P, ST, 2 * CQK], PSDT, tag="qs")
            for s in range(ST):
                for o in range(2):
                    nc.tensor.matmul(
                        qs[:, s, :],
                        xt[:, o, s * P:(s + 1) * P],
                        wsb[:, o, 0:2 * CQK],
                        start=(o == 0),
                        stop=(o == 1),
                    )
            # Q,K -> stash in one strided copy (ACT; DVE is the scarcer engine)
            nc.scalar.copy(
                qk_split(mst, CQK),
                qs.rearrange("p s (g c) -> p s g c", g=2))

            # x^T via PE transposes, drains split DVE/ACT; col 256 = ones
            ve = nc.vector if m < 6 else nc.gpsimd
            xts = xtsp.tile([P, ST, C + 1], MDT, tag="xts")
            ve.memset(xts[:, :, C:C + 1], 1.0)
            for h in range(2):
                xq = ps_xq.tile([P, 4, 2, P], MDT, tag="xq")
                for s2 in range(4):
                    s = 4 * h + s2
                    for o in range(2):
                        nc.tensor.transpose(
                            xq[:, s2, o, :], xt[:, o, s * P:(s + 1) * P],
                            ident_r)
                if h == 0:
                    nc.vector.tensor_copy(
                        xts[:, 0:4, 0:C], xq.rearrange("p s o c -> p s (o c)"))
                else:
                    nc.scalar.copy(
                        xts[:, 4:8, 0:C], xq.rearrange("p s o c -> p s (o c)"))

            # normalization chain (squares/scales on POOL, reduces on DVE).
            # For the first macros of an iteration the POOL work goes to DVE
            # instead: the collective (posted at the previous tail) occupies
            # the gpsimd queue for ~15-30us, and critical work emitted behind
            # it would stall kx two macros later.
            sq = scr.tile([P, ST, 2, CQK], MDT, tag="sq")
            ve.tensor_tensor(sq, qk_split(mst, CQK),
                             qk_split(mst, CQK), mult)
            ssq = scr.tile([P, ST, 2], F32, tag="ssq")
            nc.vector.reduce_sum(ssq, sq, axis=mybir.AxisListType.X)
            nc.scalar.sqrt(mst[:, :, CQK:CQK + 2], ssq)
            rkn = scr.tile([P, ST, 1], F32, tag="rkn")
            nc.vector.reciprocal(rkn, mst[:, :, CQK + 1:CQK + 2])
            kvt = kvb.tile([P, ST, CS], MDT, tag="kvt")
            ve.tensor_tensor(kvt[:, :, 0:CQK],
                             mst[:, :, CQK + 2:CQK + 2 + CQK],
                             rkn.to_broadcast((P, ST, CQK)), mult)
            ve.memset(kvt[:, :, CQK:CS], 1.0)

            if dbg and m in (0, 5):
                nc.sync.dma_start(dbg_xts[0 if m == 0 else 1], xts[:, :, 0:C])

            # kx accumulation runs TWO macros behind (drain/normalize slack)
            if len(prev) == 2:
                kx_mms(*prev.pop(0), xt_state)
            prev.append((m, kvt, xts))

        def kx_mms(mm, kvt_mm, xts_mm, xt_state):
            # kx_acc [33, 257] = [Kn|1]^T [xT|1]: rows 0:32 kx (row 32 x_sum),
            # col 256 k_sum (corner = count). Baseline-shaped accumulation
            # (33-col stationary, 257-col moving): small-moving-dim flipped
            # variants showed ~bf16-level per-partial noise on HW.
            kx_acc = xt_state["kx_acc"]
            for s in range(ST):
                nc.tensor.matmul(kx_acc[0:CS, 0:C + 1],
                                 kvt_mm[:, s, :], xts_mm[:, s, :],
                                 start=(mm == 0 and s == 0),
                                 stop=(mm == NM - 1 and s == ST - 1))

        def ph1_tail_post(xt_state):
            """Flush kx matmuls, post the AllReduce. No collective-dependent
            engine ops here -- they'd block the engines' in-order queues."""
            for pp in xt_state["prev"]:
                kx_mms(*pp, xt_state)
            xt_state["prev"] = []

            kx_sb = xferp.tile([CS, C + 1], F32, tag="kx_sb")
            nc.vector.tensor_copy(kx_sb, xt_state["kx_acc"][0:CS, 0:C + 1])
            cc_in = dram.tile([CS, C + 1], F32, tag="cc_in")
            cc_ga = dram.tile([2, CS, C + 1], F32, tag="cc_ga")
            nc.sync.dma_start(cc_in, kx_sb)
            if dbg:
                nc.sync.dma_start(dbg_kxsb[0:CS, 0:C + 1], kx_sb)
            if use_collective:
                # AllGather (1.0x cost) instead of AllReduce (1.875x); the
                # two halves are summed by a DMA accumulate in tail_finish.
                nc.gpsimd.collective_compute(
                    "AllGather",
                    mybir.AluOpType.bypass,
                    replica_groups=groups,
                    ins=[cc_in.opt()],
                    outs=[cc_ga.opt()],
                )
            else:
                nc.sync.dma_start(cc_ga[0], cc_in)
                nc.sync.dma_start(cc_ga[1], cc_in)
            return cc_ga

        def tail_finish(cc_ga):
            """Collective-dependent epilogue: kvp = [kx;x_sum]^T wv^T, ksum.
            Emitted `skew` macros into the NEXT iteration. The two gathered
            halves are summed by a DMA accumulate (keeps compute queues
            free of collective-dependent work)."""
            kxs_f32 = xferp.tile([CS, C + 1], F32, tag="kxs_f32")
            nc.sync.dma_start(kxs_f32, cc_ga[0])
            nc.gpsimd.dma_start(kxs_f32, cc_ga[1],
                                accum_op=mybir.AluOpType.add)
            if dbg:
                nc.sync.dma_start(dbg_cc[0:CS, 0:C + 1], kxs_f32)
            kxb = xferp.tile([CS, C], MDT, tag="kxb")
            nc.vector.tensor_copy(kxb, kxs_f32[:, 0:C])
            # transpose [kx; x_sum] -> lhsT layout [c', 33] for the kv matmul
            qt2 = ps_qt.tile([P, ST // 2, P], MDT, tag="qt_ps")
            for o in range(2):
                nc.tensor.transpose(qt2[:, o, 0:CS],
                                    kxb[:, o * P:(o + 1) * P],
                                    ident_r[0:CS, 0:CS])
            kxt = xferp.tile([P, 2, CS], MDT, tag="kxt")
            nc.vector.tensor_copy(kxt, qt2[:, 0:2, 0:CS])
            ks_d = dram.tile([CQK, 1], F32, tag="ks_d")
            nc.sync.dma_start(ks_d, kxs_f32[0:CQK, C:C + 1])
            ksum = xferp.tile([P, CS], F32, tag="ksum")
            nc.sync.dma_start(ksum[:, 0:CQK], ks_d.partition_broadcast(P))
            nc.vector.tensor_scalar_add(ksum[:, 0:CQK], ksum[:, 0:CQK], EPS)
            nc.vector.memset(ksum[:, CQK:CS], float(n_total))
            kv_ps = ps_out.tile([P, HNT], PSDT, tag="out")
            for o in range(2):
                nc.tensor.matmul(kv_ps[0:CS, 0:C], kxt[:, o, :],
                                 wsb[:, o, 2 * CQK:J],
                                 start=(o == 0), stop=(o == 1))
            kvp = xferp.tile([CS, C], MDT, tag="kvp")
            nc.vector.tensor_copy(kvp, kv_ps[0:CS, 0:C])
            if dbg:
                nc.sync.dma_start(dbg_kvp, kvp)
                nc.sync.dma_start(dbg_ksum, ksum)
            return kvp, ksum

        # ---------------- phase 2 (one macro-chunk per yield) ----------------

        def ph2_gen(stash, kvp, ksum, first):
            def chain(m):
                st_sl = stash[:, m * ST:(m + 1) * ST, 0:CS]
                prod = scr2.tile([P, ST, CS], F32, tag="prod")
                nc.gpsimd.tensor_tensor(
                    prod, st_sl,
                    ksum[:, None, :].to_broadcast((P, ST, CS)), mult)
                den = scr2.tile([P, ST, 1], F32, tag="den")
                nc.vector.reduce_sum(den, prod, axis=mybir.AxisListType.X)
                d = scr2.tile([P, ST, 1], F32, tag="d")
                nc.vector.reciprocal(d, den)
                # rows padded to 64 so transposed PAIRS land on psum
                # partitions 0 and 64. col 33 picks up ||K||*d: finite junk.
                qsc = qscp.tile([P, ST, 2 * CQK], MDT, tag="qsc")
                if first and m < 4:  # first rotation: make pad finite
                    nc.gpsimd.memset(qsc[:, :, CQK + 2:2 * CQK], 0.0)
                nc.gpsimd.tensor_tensor(
                    qsc[:, :, 0:CQK + 2],
                    stash[:, m * ST:(m + 1) * ST, 0:CQK + 2],
                    d.to_broadcast((P, ST, CQK + 2)), mult)
                return qsc

            def mms(m, qsc):
                qt_ps = ps_qt.tile([P, ST // 2, P], MDT, tag="qt_ps")
                for j in range(ST // 2):
                    pair = qsc[:, 2 * j:2 * j + 2, :].rearrange(
                        "p a b -> p (a b)")  # [128, 128]
                    nc.tensor.transpose(qt_ps[:, j, :], pair, ident_r)
                qt_sb = qtp.tile([CS, ST, P], MDT, tag="qt_sb")
                nc.vector.tensor_copy(qt_sb[:, 0::2, :],
                                      qt_ps[0:CS, :, :])
                nc.vector.tensor_copy(qt_sb[:, 1::2, :],
                                      qt_ps[2 * CQK:2 * CQK + CS, :, :])
                # 4 single-bank out matmuls; drains interleaved so the 4th
                # mm only waits on the 1st drain (pool bufs=3)
                ot = outp.tile([P, 2, NT], ODT, tag="ot")
                for i, (nh, blk) in enumerate(
                        ((0, 0), (0, 1), (1, 0), (1, 1))):
                    o_ps = ps_out.tile([P, HNT], PSDT, tag="out")
                    nc.tensor.matmul(
                        o_ps,
                        kvp[:, blk * P:(blk + 1) * P],
                        qt_sb[:, nh * (ST // 2):(nh + 1) * (ST // 2), :],
                        start=True,
                        stop=True,
                    )
                    dst = ot[:, blk, nh * HNT:(nh + 1) * HNT]
                    if i == 0:
                        nc.vector.tensor_copy(dst, o_ps)
                    else:
                        nc.scalar.copy(dst, o_ps)
                return ot

            def drains(m, ot):
                nc.sync.dma_start(out_r[:, :, m * NT:(m + 1) * NT], ot)

            mmed = []
            for m in range(NM):
                qsc = chain(m)
                mmed.append((m, qsc))
                if len(mmed) == 2:
                    mm_m, qsc_mm = mmed.pop(0)
                    drains(mm_m, mms(mm_m, qsc_mm))
                yield
            for mm_m, qsc_mm in mmed:
                drains(mm_m, mms(mm_m, qsc_mm))
            yield

        # ---------------- pipelined repeat loop ----------------
        gen = None
        gen_left = 0
        pend_cc = None
        prev_stash = None
        prev_first = False
        for it in range(repeat):
            stash = stashp.tile([P, SROW, SW], MDT, tag="stash")
            kx_acc = ps_kx.tile([P, 512], F32, tag="kx")
            xt_state = {"stash": stash, "prev": [], "kx_acc": kx_acc}
            for m in range(NM):
                ph1_macro(m, xt_state)
                if pend_cc is not None and m == skew - 1:
                    kvp, ksum = tail_finish(pend_cc)
                    pend_cc = None
                    gen = ph2_gen(prev_stash, kvp, ksum, first=prev_first)
                    gen_left = NM + 1
                    gen_ready_m = m + 2
                if gen is not None and m >= gen_ready_m:
                    slots_left = NM - 1 - m
                    pulls = 1 if gen_left <= slots_left else 2
                    for _ in range(pulls):
                        if next(gen, StopIteration) is StopIteration:
                            gen = None
                            break
                        gen_left -= 1
            cc_out = ph1_tail_post(xt_state)
            if gen is not None:  # shouldn't happen with pacing, but be safe
                for _ in gen:
                    pass
                gen = None
            if 2 in phases:
                pend_cc = cc_out
                prev_stash = stash
                prev_first = (it == 0)
            else:
                src = xs_r[:, :, 0:NT]
                if MDT != ODT:
                    src = src.bitcast(ODT)
                nc.sync.dma_start(out_r[:, :, 0:NT], src)
        if pend_cc is not None:
            kvp, ksum = tail_finish(pend_cc)
            for _ in ph2_gen(prev_stash, kvp, ksum, first=prev_first):
                pass

    nc.compile()
    return nc


_NC_CACHE = {}


def _get_nc(nsh, n_total, num_cores, groups_key, mm_dtype="bf16"):
    key = (nsh, n_total, num_cores, groups_key, mm_dtype)
    if key not in _NC_CACHE:
        groups = [list(g) for g in groups_key]
        _NC_CACHE[key] = build_attention_nc(nsh, n_total, num_cores, groups,
                                            mm_dtype)
    return _NC_CACHE[key]


def _kernel_numpy(x, wq, bq, wk, bk, wv, bv):
    """Plain numpy fallback (used only for nonzero biases / odd shapes)."""
    b, c, h, w = x.shape
    n = h * w
    xf = x.reshape(b, c, n).astype(np.float64)
    Q = np.einsum("oc,bcn->bon", wq.astype(np.float64), xf) + bq.astype(np.float64)[None, :, None]
    K = np.einsum("oc,bcn->bon", wk.astype(np.float64), xf) + bk.astype(np.float64)[None, :, None]
    V = np.einsum("oc,bcn->bon", wv.astype(np.float64), xf) + bv.astype(np.float64)[None, :, None]
    Qn = Q / np.linalg.norm(Q, axis=1, keepdims=True)
    Kn = K / np.linalg.norm(K, axis=1, keepdims=True)
    k_sum = Kn.sum(-1) + EPS
    tailor = 1.0 / (n + np.einsum("bmn,bm->bn", Qn, k_sum))
    value_sum = V.sum(-1)
    kv = np.einsum("bmn,bcn->bmc", Kn, V)
    ms = value_sum[:, :, None] + np.einsum("bmn,bmc->bcn", Qn, kv)
    return (ms * tailor[:, None, :]).reshape(b, c, h, w).astype(np.float32)


def kernel(x, wq, bq, wk, bk, wv, bv):
    x = np.asarray(x, dtype=np.float32)
    B, Cc, H, W = x.shape
    if (any(np.any(np.asarray(b_) != 0) for b_ in (bq, bk, bv))
            or Cc != C or wq.shape != (CQK, C) or wv.shape != (C, C)
            or (H * W) % (2 * NT) != 0 or B != 4):
        return _kernel_numpy(x, wq, bq, wk, bk, wv, bv)
    N = H * W
    ncores = 8
    shards_per_batch = ncores // B  # 2
    nsh = N // shards_per_batch  # 32768
    groups_key = tuple(
        tuple(range(b * shards_per_batch, (b + 1) * shards_per_batch))
        for b in range(B)
    )

    mm_dtype = "bf16"
    io_t = _np_io(mm_dtype)
    wt = np.ascontiguousarray(
        np.concatenate([np.asarray(wq).T, np.asarray(wk).T, np.asarray(wv).T],
                       axis=1).astype(io_t))

    nc = _get_nc(nsh, N, ncores, groups_key, mm_dtype)

    xr = x.reshape(B, Cc, N).astype(io_t)
    in_maps = []
    for core in range(ncores):
        b, hh = core // shards_per_batch, core % shards_per_batch
        in_maps.append({
            "xs": np.ascontiguousarray(xr[b, :, hh * nsh:(hh + 1) * nsh]),
            "wt": wt,
        })

    res = run_bass_kernel_spmd(nc, in_maps, list(range(ncores)))

    out = np.empty((B, Cc, N), np.float32)
    for core in range(ncores):
        b, hh = core // shards_per_batch, core % shards_per_batch
        out[b, :, hh * nsh:(hh + 1) * nsh] = np.asarray(
            res.results[core]["out"]).astype(np.float32)
    return out.reshape(B, Cc, H, W)
